# revision 30
# baseline (speedup 1.0000x reference)
"""LIF spiking layer (T=32, B=256, C_in=C_out=4096, fp32) on 8 trn2 NeuronCores.

Strategy: data-parallel over batch (32 samples/core, W replicated).

Matmul scheme ("hybrid", ~1.5 PE-cycles/output-col vs 3.0 for fp16 hi/lo x3):
  current*2^26 = x~ @ W~.T                                (fp16 main pass)
               + e4m3(Wl*2^9).T@e4m3(x~*2^-9)
               + e4m3(W~*2^-3).T@e4m3(xl*2^3)             (one fp8 DoubleRow
                                                           pass, 0.5 cyc/col)
  where x~ = fp16(x*2^13), W~ = fp16(W*2^13), xl/Wl the exact fp16 residuals.
  Both correction products have net scale 2^0 relative to the main psum, so
  all three accumulate into ONE psum group - no combine op. CPU-sim of this
  exact arithmetic: ~180/33.5M spike flips (rel err ~0.007, budget 2e-2).

The hi fp8 planes (x~*2^-9 and W~*2^-3) are converted on-chip by the idle
Activation engine from the fp16 tiles (saves ~25MB of HBM traffic per core);
only the residual planes (xl8, Wl8) come from the host.

LIF recurrence runs on VectorE in scaled units (th*2^26) over groups of 4
co-tiles, so each of the 3 ops/timestep covers [128, 4, 32] = 128 columns.
Psum tiles shrink over a group sequence (chunk -> quarter -> eighth) so each
tile's recurrence overlaps later matmuls and the final drain is short; psum
deps are tile-granular and a matmul group-start clears its whole psum bank's
has_written bits, so slices sharing a bank are emitted strictly sequentially
with bank alternation between consecutive slices.
"""

import numpy as np

import concourse.mybir as mybir
import concourse.tile as tile
from concourse import bacc
from concourse.bass_utils import run_bass_kernel_spmd

FP32 = mybir.dt.float32
FP16 = mybir.dt.float16
FP8 = mybir.dt.float8e4

N_CORES = 8
T, B, CI, CO = 32, 256, 4096, 4096
B_LOC = B // N_CORES  # 32
TB = T * B_LOC  # 1024

S13 = np.float32(2.0 ** 13)
SCALE = float(2.0 ** 26)  # psum units: current * 2^26

# set by test.py to collect a profile
TRACE = False
LAST_EXEC_NS = None
MODE = "hybrid"

_CACHE = {}


def build_kernel(d, th, has_bias, T=T, B_loc=B_LOC, CI=CI, CO=CO):
    TBl = T * B_loc
    n_k = CI // 128
    n_c = CO // 128
    csize = 512
    n_q = TBl // csize  # 2
    t_per_q = csize // B_loc  # 16
    GR = 4  # co-tiles per psum group
    n_g = n_c // GR
    ths = float(th) * SCALE

    nc = bacc.Bacc("TRN2", target_bir_lowering=False, debug=False, num_devices=N_CORES)

    xt = nc.declare_dram_parameter("xt", [128, n_k, TBl], FP16, isOutput=False)
    x8l = nc.declare_dram_parameter("x8l", [128, n_k, TBl], FP8, isOutput=False)
    wt = nc.declare_dram_parameter("wt", [n_c, 128, n_k, 128], FP16, isOutput=False)
    w8l = nc.declare_dram_parameter("w8l", [n_c, 128, n_k, 128], FP8, isOutput=False)
    if has_bias:
        bias = nc.declare_dram_parameter(
            "bias", [n_g, 128, GR, B_loc], FP32, isOutput=False
        )
    spk = nc.declare_dram_parameter("spk", [n_g, 128, GR, TBl], FP16, isOutput=True)

    copy_f = mybir.ActivationFunctionType.Copy

    with tile.TileContext(nc) as tc:
        with (
            tc.tile_pool(name="xp", bufs=1) as x_pool,
            tc.tile_pool(name="wp", bufs=4) as w_pool,
            tc.tile_pool(name="w8p", bufs=4) as w8_pool,
            tc.tile_pool(name="work", bufs=2) as work_pool,
            tc.tile_pool(name="pc", bufs=2, space="PSUM") as pc_pool,
        ):
            XT = x_pool.tile([128, n_k, TBl], FP16)
            # pair dim per k: plane 0 = x~8 (on-chip convert), plane 1 = xl8
            X8 = x_pool.tile([128, n_k, 2, TBl], FP8)

            # head: stream all fp16 data first (XT k-wise, the first four W
            # tiles inserted early) so group 0's k-outer fp16 matmuls track
            # the XT arrivals with all co-tiles available; then the fp8
            # residual stream. x~8/W~8 hi planes convert on ACT as their fp16
            # sources land.
            head_w = {}
            head_w8 = {}
            for c in range(GR):
                head_w[c] = w_pool.tile(
                    [128, n_k, 128], FP16, tag="wt", name=f"w{c}"
                )
                head_w8[c] = w8_pool.tile(
                    [128, 2, n_k, 128], FP8, tag="w8", name=f"w8_{c}"
                )
            nc.sync.dma_start(out=head_w[0][:, 0:4, :], in_=wt[0, :, 0:4, :])
            for k in range(n_k):
                if k == 1:
                    nc.sync.dma_start(out=head_w[1], in_=wt[1, :, :, :])
                elif k == 3:
                    nc.sync.dma_start(out=head_w[2], in_=wt[2, :, :, :])
                elif k == 5:
                    nc.sync.dma_start(out=head_w[3], in_=wt[3, :, :, :])
                if k % 4 == 0 and k > 0:
                    nc.sync.dma_start(
                        out=head_w[0][:, k : k + 4, :], in_=wt[0, :, k : k + 4, :]
                    )
                nc.sync.dma_start(out=XT[:, k, :], in_=xt[:, k, :])
                nc.scalar.activation(
                    X8[:, k, 0, :], XT[:, k, :], copy_f, scale=float(2.0 ** -9)
                )
            for c in range(GR):
                nc.sync.dma_start(out=head_w8[c][:, 0, :, :], in_=w8l[c, :, :, :])
                nc.scalar.activation(
                    head_w8[c][:, 1, :, :], head_w[c], copy_f, scale=float(2.0 ** -3)
                )
                for k in range(8 * c, 8 * c + 8):
                    nc.sync.dma_start(out=X8[:, k, 1, :], in_=x8l[:, k, :])

            for g in range(n_g):
                if g == 0:
                    Wg = {ci: (head_w[ci], head_w8[ci]) for ci in range(GR)}
                else:
                    Wg = {}
                    for ci in range(GR):
                        c = g * GR + ci
                        W_c = w_pool.tile(
                            [128, n_k, 128], FP16, tag="wt", name=f"w{c}"
                        )
                        W8_c = w8_pool.tile(
                            [128, 2, n_k, 128], FP8, tag="w8", name=f"w8_{c}"
                        )
                        nc.sync.dma_start(out=W_c, in_=wt[c, :, :, :])
                        nc.sync.dma_start(out=W8_c[:, 0, :, :], in_=w8l[c, :, :, :])
                        nc.scalar.activation(
                            W8_c[:, 1, :, :], W_c, copy_f, scale=float(2.0 ** -3)
                        )
                        Wg[ci] = (W_c, W8_c)

                if has_bias:
                    b_tile = work_pool.tile([128, GR, B_loc], FP32, tag="bt")
                    nc.sync.dma_start(out=b_tile, in_=bias[g, :, :, :])

                mem = work_pool.tile([128, GR, B_loc], FP32, tag="mem")
                nc.vector.memset(mem, 0.0)

                def rec_steps(tile_, s_tile, t0, nt):
                    for tt in range(nt):
                        oo = tt * B_loc
                        cur = tile_[:, :, oo : oo + B_loc]
                        nc.vector.scalar_tensor_tensor(
                            out=mem,
                            in0=mem,
                            scalar=d,
                            in1=cur,
                            op0=mybir.AluOpType.mult,
                            op1=mybir.AluOpType.add,
                        )
                        if has_bias:
                            nc.vector.tensor_tensor(
                                out=mem, in0=mem, in1=b_tile, op=mybir.AluOpType.add
                            )
                        s_t = s_tile[:, :, oo : oo + B_loc]
                        nc.vector.tensor_scalar(
                            s_t, mem, ths, None, mybir.AluOpType.is_gt
                        )
                        nc.vector.scalar_tensor_tensor(
                            out=mem,
                            in0=s_t,
                            scalar=-ths,
                            in1=mem,
                            op0=mybir.AluOpType.mult,
                            op1=mybir.AluOpType.add,
                        )

                if g == 0:
                    # group 0: two chunk-tiles [128, GR, 512] (each ci slice
                    # owns a full psum bank so accumulation groups interleave
                    # freely), fp16 k-outer in two ci-pair passes to track
                    # the XT DMA stream, then chunk-sequential DR + rec.
                    pcs = [
                        pc_pool.tile([128, GR, csize], FP32, tag="pc", name="pc")
                        for _ in range(n_q)
                    ]
                    # ci 0/1 track the XT stream k-wise; ci 2/3 follow LAG
                    # k-tiles behind (their W tiles arrive a few k-periods in)
                    LAG = 12
                    for kk in range(n_k + LAG):
                        for ci, k in (
                            [(0, kk), (1, kk)] if kk < n_k else []
                        ) + ([(2, kk - LAG), (3, kk - LAG)] if kk >= LAG else []):
                            for q in range(n_q):
                                nc.tensor.matmul(
                                    pcs[q][:, ci, :],
                                    lhsT=Wg[ci][0][:, k, :],
                                    rhs=XT[:, k, q * csize : (q + 1) * csize],
                                    start=(k == 0),
                                    stop=False,
                                )
                    for q in range(n_q):
                        for k in range(n_k):
                            for ci in range(GR):
                                nc.tensor.matmul(
                                    pcs[q][:, ci, :],
                                    lhsT=Wg[ci][1][:, :, k, :],
                                    rhs=X8[:, k, :, q * csize : (q + 1) * csize],
                                    start=False,
                                    stop=(k == n_k - 1),
                                    perf_mode=mybir.MatmulPerfMode.DoubleRow,
                                )
                        s_stage = work_pool.tile(
                            [128, GR, csize], FP16, tag="s", name="s_stage"
                        )
                        rec_steps(pcs[q], s_stage, q * t_per_q, t_per_q)
                        nc.sync.dma_start(
                            out=spk[g, :, :, q * csize : (q + 1) * csize],
                            in_=s_stage,
                        )
                else:
                    # groups 1+: quarter tiles [128, GR, 256] and per-slice
                    # sequential fp16+DR emission (two ci slices share a psum
                    # bank, so slices of a bank must not interleave). Each
                    # quarter's recurrence then overlaps the next quarter's
                    # matmuls, and the final group drains only ~a quarter.
                    qsize = csize // 2

                    def emit_slice(tile_, ci, off, width):
                        for k in range(n_k):
                            nc.tensor.matmul(
                                tile_[:, ci, :],
                                lhsT=Wg[ci][0][:, k, :],
                                rhs=XT[:, k, off : off + width],
                                start=(k == 0),
                                stop=False,
                            )
                        for k in range(n_k):
                            nc.tensor.matmul(
                                tile_[:, ci, :],
                                lhsT=Wg[ci][1][:, :, k, :],
                                rhs=X8[:, k, :, off : off + width],
                                start=False,
                                stop=(k == n_k - 1),
                                perf_mode=mybir.MatmulPerfMode.DoubleRow,
                            )

                    def rec_and_store(tile_, off, width):
                        s_stage = work_pool.tile(
                            [128, GR, width], FP16, tag="s", name="s_stage"
                        )
                        rec_steps(tile_, s_stage, off // B_loc, width // B_loc)
                        nc.sync.dma_start(
                            out=spk[g, :, :, off : off + width], in_=s_stage
                        )

                    n_quart = 2 * n_q if g < n_g - 1 else 2 * n_q - 1
                    for qq in range(n_quart):
                        pcq = pc_pool.tile(
                            [128, GR, qsize], FP32, tag="pc", name="pc"
                        )
                        # [128, 4, 256] spans 2 psum banks (ci 0/1 and 2/3);
                        # alternate banks between consecutive slices so a
                        # group-start's bank-wide has_written clear can never
                        # race the previous slice's in-flight accumulation
                        for ci in (0, 2, 1, 3):
                            emit_slice(pcq, ci, qq * qsize, qsize)
                        rec_and_store(pcq, qq * qsize, qsize)
                    if g == n_g - 1:
                        # final quarter as two single-bank eighth-tiles with
                        # ci interleaved across them (same bank-alternation),
                        # halving the exposed drain of the last recurrence
                        esize = qsize // 2
                        e_tiles = [
                            pc_pool.tile([128, GR, esize], FP32, tag="pc", name="pc")
                            for _ in range(2)
                        ]
                        base = (2 * n_q - 1) * qsize
                        for ci in range(GR):
                            for ei, et in enumerate(e_tiles):
                                emit_slice(et, ci, base + ei * esize, esize)
                        for ei, et in enumerate(e_tiles):
                            rec_and_store(et, base + ei * esize, esize)

    nc.compile()
    return nc


def _f8(a):
    import ml_dtypes

    return np.ascontiguousarray(a).astype(ml_dtypes.float8_e4m3)


def _xt_layout(a):
    """[TB, CI] -> [128, CI//128, TB]: partition p holds ci = k*128+p."""
    TBl, CIl = a.shape
    return np.ascontiguousarray(a.reshape(TBl, CIl // 128, 128).transpose(2, 1, 0))


def _wt_layout(Wm):
    """[CO, CI] -> [CO//128, 128, CI//128, 128]: W[c*128+j, k*128+p] at
    [c, p, k, j]."""
    COl, CIl = Wm.shape
    return np.ascontiguousarray(
        Wm.reshape(COl // 128, 128, CIl // 128, 128).transpose(0, 3, 2, 1)
    )


def kernel(x, W, b, decay, thresh):
    global LAST_EXEC_NS
    x = np.ascontiguousarray(np.asarray(x, dtype=np.float32))
    W = np.ascontiguousarray(np.asarray(W, dtype=np.float32))
    b = np.asarray(b, dtype=np.float32)
    decay = np.asarray(decay, dtype=np.float32)
    thresh = np.asarray(thresh, dtype=np.float32)

    d = float(decay.reshape(-1)[0])
    th = float(thresh.reshape(-1)[0])
    has_bias = bool(np.any(b != 0))

    key = (MODE, d, th, has_bias)
    if key not in _CACHE:
        _CACHE[key] = build_kernel(d, th, has_bias)
    nc = _CACHE[key]

    lim = np.float32(65504.0 * 0.999)

    # weights: shared across cores
    Ws = np.clip(W * S13, -lim, lim)
    Wt = Ws.astype(np.float16)
    Wl = Ws - Wt.astype(np.float32)
    wt_l = _wt_layout(Wt)
    w8l_l = _wt_layout(_f8(Wl * np.float32(2.0 ** 9)))

    in_maps = []
    n_g = (CO // 128) // 4
    for i in range(N_CORES):
        xs = x[:, i * B_LOC : (i + 1) * B_LOC, :].reshape(TB, CI)
        xs = np.clip(xs * S13, -lim, lim)
        xh = xs.astype(np.float16)
        xl = xs - xh.astype(np.float32)
        m = {
            "xt": _xt_layout(xh),
            "x8l": _xt_layout(_f8(xl * np.float32(2.0 ** 3))),
            "wt": wt_l,
            "w8l": w8l_l,
        }
        if has_bias:
            bs = (b * np.float32(SCALE)).astype(np.float32)  # [CO]
            bt = bs.reshape(n_g, 4, 128).transpose(0, 2, 1)  # [n_g, 128, 4]
            m["bias"] = np.ascontiguousarray(
                np.repeat(bt[:, :, :, None], B_LOC, axis=3).astype(np.float32)
            )
        in_maps.append(m)

    res = run_bass_kernel_spmd(
        nc, in_maps, core_ids=list(range(N_CORES)), trace=TRACE
    )
    LAST_EXEC_NS = res.exec_time_ns

    # spikes come back [n_g, 128, GR, TB] fp16 -> [T, B_loc, CO] per core
    outs = []
    for r in res.results:
        s = r["spk"]  # [n_g, 128, GR, TB]
        s = (
            s.reshape(n_g, 128, 4, T, B_LOC)
            .transpose(3, 4, 0, 2, 1)
            .reshape(T, B_LOC, CO)
        )
        outs.append(s.astype(np.float32))
    out = np.concatenate(outs, axis=1)
    return np.ascontiguousarray(out)


# revision 31
# speedup vs baseline: 1.0002x; 1.0002x over previous
"""LIF spiking layer (T=32, B=256, C_in=C_out=4096, fp32) on 8 trn2 NeuronCores.

Strategy: data-parallel over batch (32 samples/core, W replicated).

Matmul scheme ("hybrid", ~1.5 PE-cycles/output-col vs 3.0 for fp16 hi/lo x3):
  current*2^26 = x~ @ W~.T                                (fp16 main pass)
               + e4m3(Wl*2^9).T@e4m3(x~*2^-9)
               + e4m3(W~*2^-3).T@e4m3(xl*2^3)             (one fp8 DoubleRow
                                                           pass, 0.5 cyc/col)
  where x~ = fp16(x*2^13), W~ = fp16(W*2^13), xl/Wl the exact fp16 residuals.
  Both correction products have net scale 2^0 relative to the main psum, so
  all three accumulate into ONE psum group - no combine op. CPU-sim of this
  exact arithmetic: ~180/33.5M spike flips (rel err ~0.007, budget 2e-2).

The hi fp8 planes (x~*2^-9 and W~*2^-3) are converted on-chip by the idle
Activation engine from the fp16 tiles (saves ~25MB of HBM traffic per core);
only the residual planes (xl8, Wl8) come from the host.

LIF recurrence runs on VectorE in scaled units (th*2^26) over groups of 4
co-tiles, so each of the 3 ops/timestep covers [128, 4, 32] = 128 columns.
Psum tiles shrink over a group sequence (chunk -> quarter -> eighth) so each
tile's recurrence overlaps later matmuls and the final drain is short; psum
deps are tile-granular and a matmul group-start clears its whole psum bank's
has_written bits, so slices sharing a bank are emitted strictly sequentially
with bank alternation between consecutive slices.
"""

import numpy as np

import concourse.mybir as mybir
import concourse.tile as tile
from concourse import bacc
from concourse.bass_utils import run_bass_kernel_spmd

FP32 = mybir.dt.float32
FP16 = mybir.dt.float16
FP8 = mybir.dt.float8e4

N_CORES = 8
T, B, CI, CO = 32, 256, 4096, 4096
B_LOC = B // N_CORES  # 32
TB = T * B_LOC  # 1024

S13 = np.float32(2.0 ** 13)
SCALE = float(2.0 ** 26)  # psum units: current * 2^26

# set by test.py to collect a profile
TRACE = False
LAST_EXEC_NS = None
MODE = "hybrid"

_CACHE = {}


def build_kernel(d, th, has_bias, T=T, B_loc=B_LOC, CI=CI, CO=CO):
    TBl = T * B_loc
    n_k = CI // 128
    n_c = CO // 128
    csize = 512
    n_q = TBl // csize  # 2
    t_per_q = csize // B_loc  # 16
    GR = 4  # co-tiles per psum group
    n_g = n_c // GR
    ths = float(th) * SCALE

    nc = bacc.Bacc("TRN2", target_bir_lowering=False, debug=False, num_devices=N_CORES)

    xt = nc.declare_dram_parameter("xt", [128, n_k, TBl], FP16, isOutput=False)
    x8l = nc.declare_dram_parameter("x8l", [128, n_k, TBl], FP8, isOutput=False)
    wt = nc.declare_dram_parameter("wt", [n_c, 128, n_k, 128], FP16, isOutput=False)
    w8l = nc.declare_dram_parameter("w8l", [n_c, 128, n_k, 128], FP8, isOutput=False)
    if has_bias:
        bias = nc.declare_dram_parameter(
            "bias", [n_g, 128, GR, B_loc], FP32, isOutput=False
        )
    spk = nc.declare_dram_parameter("spk", [n_g, 128, GR, TBl], FP16, isOutput=True)

    copy_f = mybir.ActivationFunctionType.Copy

    with tile.TileContext(nc) as tc:
        with (
            tc.tile_pool(name="xp", bufs=1) as x_pool,
            tc.tile_pool(name="wp", bufs=4) as w_pool,
            tc.tile_pool(name="w8p", bufs=4) as w8_pool,
            tc.tile_pool(name="work", bufs=2) as work_pool,
            tc.tile_pool(name="pc", bufs=2, space="PSUM") as pc_pool,
        ):
            XT = x_pool.tile([128, n_k, TBl], FP16)
            # pair dim per k: plane 0 = x~8 (on-chip convert), plane 1 = xl8
            X8 = x_pool.tile([128, n_k, 2, TBl], FP8)

            # head: stream all fp16 data first (XT k-wise, the first four W
            # tiles inserted early) so group 0's k-outer fp16 matmuls track
            # the XT arrivals with all co-tiles available; then the fp8
            # residual stream. x~8/W~8 hi planes convert on ACT as their fp16
            # sources land.
            head_w = {}
            head_w8 = {}
            for c in range(GR):
                head_w[c] = w_pool.tile(
                    [128, n_k, 128], FP16, tag="wt", name=f"w{c}"
                )
                head_w8[c] = w8_pool.tile(
                    [128, 2, n_k, 128], FP8, tag="w8", name=f"w8_{c}"
                )
            nc.sync.dma_start(out=head_w[0][:, 0:4, :], in_=wt[0, :, 0:4, :])
            for k in range(n_k):
                if k == 1:
                    nc.sync.dma_start(out=head_w[1], in_=wt[1, :, :, :])
                elif k == 7:
                    nc.sync.dma_start(out=head_w[2], in_=wt[2, :, :, :])
                elif k == 9:
                    nc.sync.dma_start(out=head_w[3], in_=wt[3, :, :, :])
                if k % 4 == 0 and k > 0:
                    nc.sync.dma_start(
                        out=head_w[0][:, k : k + 4, :], in_=wt[0, :, k : k + 4, :]
                    )
                nc.sync.dma_start(out=XT[:, k, :], in_=xt[:, k, :])
                nc.scalar.activation(
                    X8[:, k, 0, :], XT[:, k, :], copy_f, scale=float(2.0 ** -9)
                )
            for c in range(GR):
                nc.sync.dma_start(out=head_w8[c][:, 0, :, :], in_=w8l[c, :, :, :])
                nc.scalar.activation(
                    head_w8[c][:, 1, :, :], head_w[c], copy_f, scale=float(2.0 ** -3)
                )
                for k in range(8 * c, 8 * c + 8):
                    nc.sync.dma_start(out=X8[:, k, 1, :], in_=x8l[:, k, :])

            for g in range(n_g):
                if g == 0:
                    Wg = {ci: (head_w[ci], head_w8[ci]) for ci in range(GR)}
                else:
                    Wg = {}
                    for ci in range(GR):
                        c = g * GR + ci
                        W_c = w_pool.tile(
                            [128, n_k, 128], FP16, tag="wt", name=f"w{c}"
                        )
                        W8_c = w8_pool.tile(
                            [128, 2, n_k, 128], FP8, tag="w8", name=f"w8_{c}"
                        )
                        nc.sync.dma_start(out=W_c, in_=wt[c, :, :, :])
                        nc.sync.dma_start(out=W8_c[:, 0, :, :], in_=w8l[c, :, :, :])
                        nc.scalar.activation(
                            W8_c[:, 1, :, :], W_c, copy_f, scale=float(2.0 ** -3)
                        )
                        Wg[ci] = (W_c, W8_c)

                if has_bias:
                    b_tile = work_pool.tile([128, GR, B_loc], FP32, tag="bt")
                    nc.sync.dma_start(out=b_tile, in_=bias[g, :, :, :])

                mem = work_pool.tile([128, GR, B_loc], FP32, tag="mem")
                nc.vector.memset(mem, 0.0)

                def rec_steps(tile_, s_tile, t0, nt):
                    for tt in range(nt):
                        oo = tt * B_loc
                        cur = tile_[:, :, oo : oo + B_loc]
                        nc.vector.scalar_tensor_tensor(
                            out=mem,
                            in0=mem,
                            scalar=d,
                            in1=cur,
                            op0=mybir.AluOpType.mult,
                            op1=mybir.AluOpType.add,
                        )
                        if has_bias:
                            nc.vector.tensor_tensor(
                                out=mem, in0=mem, in1=b_tile, op=mybir.AluOpType.add
                            )
                        s_t = s_tile[:, :, oo : oo + B_loc]
                        nc.vector.tensor_scalar(
                            s_t, mem, ths, None, mybir.AluOpType.is_gt
                        )
                        nc.vector.scalar_tensor_tensor(
                            out=mem,
                            in0=s_t,
                            scalar=-ths,
                            in1=mem,
                            op0=mybir.AluOpType.mult,
                            op1=mybir.AluOpType.add,
                        )

                if g == 0:
                    # group 0: two chunk-tiles [128, GR, 512] (each ci slice
                    # owns a full psum bank so accumulation groups interleave
                    # freely), fp16 k-outer in two ci-pair passes to track
                    # the XT DMA stream, then chunk-sequential DR + rec.
                    pcs = [
                        pc_pool.tile([128, GR, csize], FP32, tag="pc", name="pc")
                        for _ in range(n_q)
                    ]
                    # ci 0/1 track the XT stream k-wise; ci 2/3 follow LAG
                    # k-tiles behind (their W tiles arrive a few k-periods in)
                    LAG = 12
                    for kk in range(n_k + LAG):
                        for ci, k in (
                            [(0, kk), (1, kk)] if kk < n_k else []
                        ) + ([(2, kk - LAG), (3, kk - LAG)] if kk >= LAG else []):
                            for q in range(n_q):
                                nc.tensor.matmul(
                                    pcs[q][:, ci, :],
                                    lhsT=Wg[ci][0][:, k, :],
                                    rhs=XT[:, k, q * csize : (q + 1) * csize],
                                    start=(k == 0),
                                    stop=False,
                                )
                    for q in range(n_q):
                        for k in range(n_k):
                            for ci in range(GR):
                                nc.tensor.matmul(
                                    pcs[q][:, ci, :],
                                    lhsT=Wg[ci][1][:, :, k, :],
                                    rhs=X8[:, k, :, q * csize : (q + 1) * csize],
                                    start=False,
                                    stop=(k == n_k - 1),
                                    perf_mode=mybir.MatmulPerfMode.DoubleRow,
                                )
                        s_stage = work_pool.tile(
                            [128, GR, csize], FP16, tag="s", name="s_stage"
                        )
                        rec_steps(pcs[q], s_stage, q * t_per_q, t_per_q)
                        nc.sync.dma_start(
                            out=spk[g, :, :, q * csize : (q + 1) * csize],
                            in_=s_stage,
                        )
                else:
                    # groups 1+: quarter tiles [128, GR, 256] and per-slice
                    # sequential fp16+DR emission (two ci slices share a psum
                    # bank, so slices of a bank must not interleave). Each
                    # quarter's recurrence then overlaps the next quarter's
                    # matmuls, and the final group drains only ~a quarter.
                    qsize = csize // 2

                    def emit_slice(tile_, ci, off, width):
                        for k in range(n_k):
                            nc.tensor.matmul(
                                tile_[:, ci, :],
                                lhsT=Wg[ci][0][:, k, :],
                                rhs=XT[:, k, off : off + width],
                                start=(k == 0),
                                stop=False,
                            )
                        for k in range(n_k):
                            nc.tensor.matmul(
                                tile_[:, ci, :],
                                lhsT=Wg[ci][1][:, :, k, :],
                                rhs=X8[:, k, :, off : off + width],
                                start=False,
                                stop=(k == n_k - 1),
                                perf_mode=mybir.MatmulPerfMode.DoubleRow,
                            )

                    def rec_and_store(tile_, off, width):
                        s_stage = work_pool.tile(
                            [128, GR, width], FP16, tag="s", name="s_stage"
                        )
                        rec_steps(tile_, s_stage, off // B_loc, width // B_loc)
                        nc.sync.dma_start(
                            out=spk[g, :, :, off : off + width], in_=s_stage
                        )

                    n_quart = 2 * n_q if g < n_g - 1 else 2 * n_q - 1
                    for qq in range(n_quart):
                        pcq = pc_pool.tile(
                            [128, GR, qsize], FP32, tag="pc", name="pc"
                        )
                        # [128, 4, 256] spans 2 psum banks (ci 0/1 and 2/3);
                        # alternate banks between consecutive slices so a
                        # group-start's bank-wide has_written clear can never
                        # race the previous slice's in-flight accumulation
                        for ci in (0, 2, 1, 3):
                            emit_slice(pcq, ci, qq * qsize, qsize)
                        rec_and_store(pcq, qq * qsize, qsize)
                    if g == n_g - 1:
                        # final quarter as two single-bank eighth-tiles with
                        # ci interleaved across them (same bank-alternation),
                        # halving the exposed drain of the last recurrence
                        esize = qsize // 2
                        e_tiles = [
                            pc_pool.tile([128, GR, esize], FP32, tag="pc", name="pc")
                            for _ in range(2)
                        ]
                        base = (2 * n_q - 1) * qsize
                        for ci in range(GR):
                            for ei, et in enumerate(e_tiles):
                                emit_slice(et, ci, base + ei * esize, esize)
                        for ei, et in enumerate(e_tiles):
                            rec_and_store(et, base + ei * esize, esize)

    nc.compile()
    return nc


def _f8(a):
    import ml_dtypes

    return np.ascontiguousarray(a).astype(ml_dtypes.float8_e4m3)


def _xt_layout(a):
    """[TB, CI] -> [128, CI//128, TB]: partition p holds ci = k*128+p."""
    TBl, CIl = a.shape
    return np.ascontiguousarray(a.reshape(TBl, CIl // 128, 128).transpose(2, 1, 0))


def _wt_layout(Wm):
    """[CO, CI] -> [CO//128, 128, CI//128, 128]: W[c*128+j, k*128+p] at
    [c, p, k, j]."""
    COl, CIl = Wm.shape
    return np.ascontiguousarray(
        Wm.reshape(COl // 128, 128, CIl // 128, 128).transpose(0, 3, 2, 1)
    )


def kernel(x, W, b, decay, thresh):
    global LAST_EXEC_NS
    x = np.ascontiguousarray(np.asarray(x, dtype=np.float32))
    W = np.ascontiguousarray(np.asarray(W, dtype=np.float32))
    b = np.asarray(b, dtype=np.float32)
    decay = np.asarray(decay, dtype=np.float32)
    thresh = np.asarray(thresh, dtype=np.float32)

    d = float(decay.reshape(-1)[0])
    th = float(thresh.reshape(-1)[0])
    has_bias = bool(np.any(b != 0))

    key = (MODE, d, th, has_bias)
    if key not in _CACHE:
        _CACHE[key] = build_kernel(d, th, has_bias)
    nc = _CACHE[key]

    lim = np.float32(65504.0 * 0.999)

    # weights: shared across cores
    Ws = np.clip(W * S13, -lim, lim)
    Wt = Ws.astype(np.float16)
    Wl = Ws - Wt.astype(np.float32)
    wt_l = _wt_layout(Wt)
    w8l_l = _wt_layout(_f8(Wl * np.float32(2.0 ** 9)))

    in_maps = []
    n_g = (CO // 128) // 4
    for i in range(N_CORES):
        xs = x[:, i * B_LOC : (i + 1) * B_LOC, :].reshape(TB, CI)
        xs = np.clip(xs * S13, -lim, lim)
        xh = xs.astype(np.float16)
        xl = xs - xh.astype(np.float32)
        m = {
            "xt": _xt_layout(xh),
            "x8l": _xt_layout(_f8(xl * np.float32(2.0 ** 3))),
            "wt": wt_l,
            "w8l": w8l_l,
        }
        if has_bias:
            bs = (b * np.float32(SCALE)).astype(np.float32)  # [CO]
            bt = bs.reshape(n_g, 4, 128).transpose(0, 2, 1)  # [n_g, 128, 4]
            m["bias"] = np.ascontiguousarray(
                np.repeat(bt[:, :, :, None], B_LOC, axis=3).astype(np.float32)
            )
        in_maps.append(m)

    res = run_bass_kernel_spmd(
        nc, in_maps, core_ids=list(range(N_CORES)), trace=TRACE
    )
    LAST_EXEC_NS = res.exec_time_ns

    # spikes come back [n_g, 128, GR, TB] fp16 -> [T, B_loc, CO] per core
    outs = []
    for r in res.results:
        s = r["spk"]  # [n_g, 128, GR, TB]
        s = (
            s.reshape(n_g, 128, 4, T, B_LOC)
            .transpose(3, 4, 0, 2, 1)
            .reshape(T, B_LOC, CO)
        )
        outs.append(s.astype(np.float32))
    out = np.concatenate(outs, axis=1)
    return np.ascontiguousarray(out)


# revision 32
# speedup vs baseline: 1.0022x; 1.0020x over previous
"""LIF spiking layer (T=32, B=256, C_in=C_out=4096, fp32) on 8 trn2 NeuronCores.

Strategy: data-parallel over batch (32 samples/core, W replicated).

Matmul scheme ("hybrid", ~1.5 PE-cycles/output-col vs 3.0 for fp16 hi/lo x3):
  current*2^26 = x~ @ W~.T                                (fp16 main pass)
               + e4m3(Wl*2^9).T@e4m3(x~*2^-9)
               + e4m3(W~*2^-3).T@e4m3(xl*2^3)             (one fp8 DoubleRow
                                                           pass, 0.5 cyc/col)
  where x~ = fp16(x*2^13), W~ = fp16(W*2^13), xl/Wl the exact fp16 residuals.
  Both correction products have net scale 2^0 relative to the main psum, so
  all three accumulate into ONE psum group - no combine op. CPU-sim of this
  exact arithmetic: ~180/33.5M spike flips (rel err ~0.007, budget 2e-2).

The hi fp8 planes (x~*2^-9 and W~*2^-3) are converted on-chip by the idle
Activation engine from the fp16 tiles (saves ~25MB of HBM traffic per core);
only the residual planes (xl8, Wl8) come from the host.

LIF recurrence runs on VectorE in scaled units (th*2^26) over groups of 4
co-tiles, so each of the 3 ops/timestep covers [128, 4, 32] = 128 columns.
Psum tiles shrink over a group sequence (chunk -> quarter -> eighth) so each
tile's recurrence overlaps later matmuls and the final drain is short; psum
deps are tile-granular and a matmul group-start clears its whole psum bank's
has_written bits, so slices sharing a bank are emitted strictly sequentially
with bank alternation between consecutive slices.
"""

import numpy as np

import concourse.mybir as mybir
import concourse.tile as tile
from concourse import bacc
from concourse.bass_utils import run_bass_kernel_spmd

FP32 = mybir.dt.float32
FP16 = mybir.dt.float16
FP8 = mybir.dt.float8e4

N_CORES = 8
T, B, CI, CO = 32, 256, 4096, 4096
B_LOC = B // N_CORES  # 32
TB = T * B_LOC  # 1024

S13 = np.float32(2.0 ** 13)
SCALE = float(2.0 ** 26)  # psum units: current * 2^26

# set by test.py to collect a profile
TRACE = False
LAST_EXEC_NS = None
MODE = "hybrid"

_CACHE = {}


def build_kernel(d, th, has_bias, T=T, B_loc=B_LOC, CI=CI, CO=CO):
    TBl = T * B_loc
    n_k = CI // 128
    n_c = CO // 128
    csize = 512
    n_q = TBl // csize  # 2
    t_per_q = csize // B_loc  # 16
    GR = 4  # co-tiles per psum group
    n_g = n_c // GR
    ths = float(th) * SCALE

    nc = bacc.Bacc("TRN2", target_bir_lowering=False, debug=False, num_devices=N_CORES)

    xt = nc.declare_dram_parameter("xt", [128, n_k, TBl], FP16, isOutput=False)
    x8l = nc.declare_dram_parameter("x8l", [128, n_k, TBl], FP8, isOutput=False)
    wt = nc.declare_dram_parameter("wt", [n_c, 128, n_k, 128], FP16, isOutput=False)
    w8l = nc.declare_dram_parameter("w8l", [n_c, 128, n_k, 128], FP8, isOutput=False)
    if has_bias:
        bias = nc.declare_dram_parameter(
            "bias", [n_g, 128, GR, B_loc], FP32, isOutput=False
        )
    spk = nc.declare_dram_parameter("spk", [n_g, 128, GR, TBl], FP16, isOutput=True)

    copy_f = mybir.ActivationFunctionType.Copy

    with tile.TileContext(nc) as tc:
        with (
            tc.tile_pool(name="xp", bufs=1) as x_pool,
            tc.tile_pool(name="wp", bufs=4) as w_pool,
            tc.tile_pool(name="w8p", bufs=4) as w8_pool,
            tc.tile_pool(name="work", bufs=2) as work_pool,
            tc.tile_pool(name="pc", bufs=2, space="PSUM") as pc_pool,
        ):
            XT = x_pool.tile([128, n_k, TBl], FP16)
            # pair dim per k: plane 0 = x~8 (on-chip convert), plane 1 = xl8
            X8 = x_pool.tile([128, n_k, 2, TBl], FP8)

            # head: stream all fp16 data first (XT k-wise, the first four W
            # tiles inserted early) so group 0's k-outer fp16 matmuls track
            # the XT arrivals with all co-tiles available; then the fp8
            # residual stream. x~8/W~8 hi planes convert on ACT as their fp16
            # sources land.
            head_w = {}
            head_w8 = {}
            for c in range(GR):
                head_w[c] = w_pool.tile(
                    [128, n_k, 128], FP16, tag="wt", name=f"w{c}"
                )
                head_w8[c] = w8_pool.tile(
                    [128, 2, n_k, 128], FP8, tag="w8", name=f"w8_{c}"
                )
            nc.sync.dma_start(out=head_w[0][:, 0:4, :], in_=wt[0, :, 0:4, :])
            for k in range(n_k):
                if k == 1:
                    nc.sync.dma_start(out=head_w[1], in_=wt[1, :, :, :])
                elif k == 7:
                    nc.sync.dma_start(out=head_w[2], in_=wt[2, :, :, :])
                elif k == 9:
                    nc.sync.dma_start(out=head_w[3], in_=wt[3, :, :, :])
                if k % 4 == 0 and k > 0:
                    nc.sync.dma_start(
                        out=head_w[0][:, k : k + 4, :], in_=wt[0, :, k : k + 4, :]
                    )
                nc.sync.dma_start(out=XT[:, k, :], in_=xt[:, k, :])
                nc.scalar.activation(
                    X8[:, k, 0, :], XT[:, k, :], copy_f, scale=float(2.0 ** -9)
                )
            for c in range(GR):
                nc.sync.dma_start(out=head_w8[c][:, 0, :, :], in_=w8l[c, :, :, :])
                nc.scalar.activation(
                    head_w8[c][:, 1, :, :], head_w[c], copy_f, scale=float(2.0 ** -3)
                )
                for k in range(8 * c, 8 * c + 8):
                    nc.sync.dma_start(out=X8[:, k, 1, :], in_=x8l[:, k, :])

            for g in range(n_g):
                if g == 0:
                    Wg = {ci: (head_w[ci], head_w8[ci]) for ci in range(GR)}
                else:
                    Wg = {}
                    for ci in range(GR):
                        c = g * GR + ci
                        W_c = w_pool.tile(
                            [128, n_k, 128], FP16, tag="wt", name=f"w{c}"
                        )
                        W8_c = w8_pool.tile(
                            [128, 2, n_k, 128], FP8, tag="w8", name=f"w8_{c}"
                        )
                        nc.sync.dma_start(out=W_c, in_=wt[c, :, :, :])
                        nc.sync.dma_start(out=W8_c[:, 0, :, :], in_=w8l[c, :, :, :])
                        nc.scalar.activation(
                            W8_c[:, 1, :, :], W_c, copy_f, scale=float(2.0 ** -3)
                        )
                        Wg[ci] = (W_c, W8_c)

                if has_bias:
                    b_tile = work_pool.tile([128, GR, B_loc], FP32, tag="bt")
                    nc.sync.dma_start(out=b_tile, in_=bias[g, :, :, :])

                mem = work_pool.tile([128, GR, B_loc], FP32, tag="mem")
                nc.vector.memset(mem, 0.0)

                def rec_steps(tile_, s_tile, t0, nt):
                    for tt in range(nt):
                        oo = tt * B_loc
                        cur = tile_[:, :, oo : oo + B_loc]
                        nc.vector.scalar_tensor_tensor(
                            out=mem,
                            in0=mem,
                            scalar=d,
                            in1=cur,
                            op0=mybir.AluOpType.mult,
                            op1=mybir.AluOpType.add,
                        )
                        if has_bias:
                            nc.vector.tensor_tensor(
                                out=mem, in0=mem, in1=b_tile, op=mybir.AluOpType.add
                            )
                        s_t = s_tile[:, :, oo : oo + B_loc]
                        nc.vector.tensor_scalar(
                            s_t, mem, ths, None, mybir.AluOpType.is_gt
                        )
                        nc.vector.scalar_tensor_tensor(
                            out=mem,
                            in0=s_t,
                            scalar=-ths,
                            in1=mem,
                            op0=mybir.AluOpType.mult,
                            op1=mybir.AluOpType.add,
                        )

                if g == 0:
                    # group 0: two chunk-tiles [128, GR, 512] (each ci slice
                    # owns a full psum bank so accumulation groups interleave
                    # freely), fp16 k-outer in two ci-pair passes to track
                    # the XT DMA stream, then chunk-sequential DR + rec.
                    pcs = [
                        pc_pool.tile([128, GR, csize], FP32, tag="pc", name="pc")
                        for _ in range(n_q)
                    ]
                    # ci 0/1 track the XT stream k-wise; ci 2/3 follow LAG
                    # k-tiles behind (their W tiles arrive a few k-periods in)
                    LAG = 12
                    for kk in range(n_k + LAG):
                        for ci, k in (
                            [(0, kk), (1, kk)] if kk < n_k else []
                        ) + ([(2, kk - LAG), (3, kk - LAG)] if kk >= LAG else []):
                            for q in range(n_q):
                                nc.tensor.matmul(
                                    pcs[q][:, ci, :],
                                    lhsT=Wg[ci][0][:, k, :],
                                    rhs=XT[:, k, q * csize : (q + 1) * csize],
                                    start=(k == 0),
                                    stop=False,
                                )
                    for q in range(n_q):
                        for k in range(n_k):
                            for ci in range(GR):
                                nc.tensor.matmul(
                                    pcs[q][:, ci, :],
                                    lhsT=Wg[ci][1][:, :, k, :],
                                    rhs=X8[:, k, :, q * csize : (q + 1) * csize],
                                    start=False,
                                    stop=(k == n_k - 1),
                                    perf_mode=mybir.MatmulPerfMode.DoubleRow,
                                )
                        s_stage = work_pool.tile(
                            [128, GR, csize], FP16, tag="s", name="s_stage"
                        )
                        rec_steps(pcs[q], s_stage, q * t_per_q, t_per_q)
                        nc.sync.dma_start(
                            out=spk[g, :, :, q * csize : (q + 1) * csize],
                            in_=s_stage,
                        )
                else:
                    # groups 1+: quarter tiles [128, GR, 256] and per-slice
                    # sequential fp16+DR emission (two ci slices share a psum
                    # bank, so slices of a bank must not interleave). Each
                    # quarter's recurrence then overlaps the next quarter's
                    # matmuls, and the final group drains only ~a quarter.
                    qsize = csize // 2

                    def emit_slice(tile_, ci, off, width):
                        for k in range(n_k):
                            nc.tensor.matmul(
                                tile_[:, ci, :],
                                lhsT=Wg[ci][0][:, k, :],
                                rhs=XT[:, k, off : off + width],
                                start=(k == 0),
                                stop=False,
                            )
                        for k in range(n_k):
                            nc.tensor.matmul(
                                tile_[:, ci, :],
                                lhsT=Wg[ci][1][:, :, k, :],
                                rhs=X8[:, k, :, off : off + width],
                                start=False,
                                stop=(k == n_k - 1),
                                perf_mode=mybir.MatmulPerfMode.DoubleRow,
                            )

                    def rec_and_store(tile_, off, width):
                        s_stage = work_pool.tile(
                            [128, GR, width], FP16, tag="s", name="s_stage"
                        )
                        rec_steps(tile_, s_stage, off // B_loc, width // B_loc)
                        nc.sync.dma_start(
                            out=spk[g, :, :, off : off + width], in_=s_stage
                        )

                    n_quart = 2 * n_q if g < n_g - 1 else 2 * n_q - 1
                    for qq in range(n_quart):
                        pcq = pc_pool.tile(
                            [128, GR, qsize], FP32, tag="pc", name="pc"
                        )
                        # [128, 4, 256] spans 2 psum banks (ci 0/1 and 2/3);
                        # alternate banks between consecutive slices so a
                        # group-start's bank-wide has_written clear can never
                        # race the previous slice's in-flight accumulation
                        for ci in (0, 2, 1, 3):
                            emit_slice(pcq, ci, qq * qsize, qsize)
                        rec_and_store(pcq, qq * qsize, qsize)
                    if g == n_g - 1:
                        # final quarter as two single-bank eighth-tiles with
                        # ci interleaved across them (same bank-alternation),
                        # halving the exposed drain of the last recurrence
                        widths = [qsize // 2, qsize // 4, qsize // 4]
                        offs = [0, qsize // 2, 3 * qsize // 4]
                        base = (2 * n_q - 1) * qsize
                        e_tiles = [
                            pc_pool.tile([128, GR, w], FP32, tag="pc", name="pc")
                            for w in widths
                        ]
                        for ci in range(GR):
                            for et, off, w in zip(e_tiles, offs, widths):
                                emit_slice(et, ci, base + off, w)
                        for et, off, w in zip(e_tiles, offs, widths):
                            rec_and_store(et, base + off, w)

    nc.compile()
    return nc


def _f8(a):
    import ml_dtypes

    return np.ascontiguousarray(a).astype(ml_dtypes.float8_e4m3)


def _xt_layout(a):
    """[TB, CI] -> [128, CI//128, TB]: partition p holds ci = k*128+p."""
    TBl, CIl = a.shape
    return np.ascontiguousarray(a.reshape(TBl, CIl // 128, 128).transpose(2, 1, 0))


def _wt_layout(Wm):
    """[CO, CI] -> [CO//128, 128, CI//128, 128]: W[c*128+j, k*128+p] at
    [c, p, k, j]."""
    COl, CIl = Wm.shape
    return np.ascontiguousarray(
        Wm.reshape(COl // 128, 128, CIl // 128, 128).transpose(0, 3, 2, 1)
    )


def kernel(x, W, b, decay, thresh):
    global LAST_EXEC_NS
    x = np.ascontiguousarray(np.asarray(x, dtype=np.float32))
    W = np.ascontiguousarray(np.asarray(W, dtype=np.float32))
    b = np.asarray(b, dtype=np.float32)
    decay = np.asarray(decay, dtype=np.float32)
    thresh = np.asarray(thresh, dtype=np.float32)

    d = float(decay.reshape(-1)[0])
    th = float(thresh.reshape(-1)[0])
    has_bias = bool(np.any(b != 0))

    key = (MODE, d, th, has_bias)
    if key not in _CACHE:
        _CACHE[key] = build_kernel(d, th, has_bias)
    nc = _CACHE[key]

    lim = np.float32(65504.0 * 0.999)

    # weights: shared across cores
    Ws = np.clip(W * S13, -lim, lim)
    Wt = Ws.astype(np.float16)
    Wl = Ws - Wt.astype(np.float32)
    wt_l = _wt_layout(Wt)
    w8l_l = _wt_layout(_f8(Wl * np.float32(2.0 ** 9)))

    in_maps = []
    n_g = (CO // 128) // 4
    for i in range(N_CORES):
        xs = x[:, i * B_LOC : (i + 1) * B_LOC, :].reshape(TB, CI)
        xs = np.clip(xs * S13, -lim, lim)
        xh = xs.astype(np.float16)
        xl = xs - xh.astype(np.float32)
        m = {
            "xt": _xt_layout(xh),
            "x8l": _xt_layout(_f8(xl * np.float32(2.0 ** 3))),
            "wt": wt_l,
            "w8l": w8l_l,
        }
        if has_bias:
            bs = (b * np.float32(SCALE)).astype(np.float32)  # [CO]
            bt = bs.reshape(n_g, 4, 128).transpose(0, 2, 1)  # [n_g, 128, 4]
            m["bias"] = np.ascontiguousarray(
                np.repeat(bt[:, :, :, None], B_LOC, axis=3).astype(np.float32)
            )
        in_maps.append(m)

    res = run_bass_kernel_spmd(
        nc, in_maps, core_ids=list(range(N_CORES)), trace=TRACE
    )
    LAST_EXEC_NS = res.exec_time_ns

    # spikes come back [n_g, 128, GR, TB] fp16 -> [T, B_loc, CO] per core
    outs = []
    for r in res.results:
        s = r["spk"]  # [n_g, 128, GR, TB]
        s = (
            s.reshape(n_g, 128, 4, T, B_LOC)
            .transpose(3, 4, 0, 2, 1)
            .reshape(T, B_LOC, CO)
        )
        outs.append(s.astype(np.float32))
    out = np.concatenate(outs, axis=1)
    return np.ascontiguousarray(out)


# revision 33
# speedup vs baseline: 1.0030x; 1.0008x over previous
"""LIF spiking layer (T=32, B=256, C_in=C_out=4096, fp32) on 8 trn2 NeuronCores.

Strategy: data-parallel over batch (32 samples/core, W replicated).

Matmul scheme ("hybrid", ~1.5 PE-cycles/output-col vs 3.0 for fp16 hi/lo x3):
  current*2^26 = x~ @ W~.T                                (fp16 main pass)
               + e4m3(Wl*2^9).T@e4m3(x~*2^-9)
               + e4m3(W~*2^-3).T@e4m3(xl*2^3)             (one fp8 DoubleRow
                                                           pass, 0.5 cyc/col)
  where x~ = fp16(x*2^13), W~ = fp16(W*2^13), xl/Wl the exact fp16 residuals.
  Both correction products have net scale 2^0 relative to the main psum, so
  all three accumulate into ONE psum group - no combine op. CPU-sim of this
  exact arithmetic: ~180/33.5M spike flips (rel err ~0.007, budget 2e-2).

The hi fp8 planes (x~*2^-9 and W~*2^-3) are converted on-chip by the idle
Activation engine from the fp16 tiles (saves ~25MB of HBM traffic per core);
only the residual planes (xl8, Wl8) come from the host.

LIF recurrence runs on VectorE in scaled units (th*2^26) over groups of 4
co-tiles, so each of the 3 ops/timestep covers [128, 4, 32] = 128 columns.
Psum tiles shrink over a group sequence (chunk -> quarter -> eighth) so each
tile's recurrence overlaps later matmuls and the final drain is short; psum
deps are tile-granular and a matmul group-start clears its whole psum bank's
has_written bits, so slices sharing a bank are emitted strictly sequentially
with bank alternation between consecutive slices.
"""

import numpy as np

import concourse.mybir as mybir
import concourse.tile as tile
from concourse import bacc
from concourse.bass_utils import run_bass_kernel_spmd

FP32 = mybir.dt.float32
FP16 = mybir.dt.float16
FP8 = mybir.dt.float8e4

N_CORES = 8
T, B, CI, CO = 32, 256, 4096, 4096
B_LOC = B // N_CORES  # 32
TB = T * B_LOC  # 1024

S13 = np.float32(2.0 ** 13)
SCALE = float(2.0 ** 26)  # psum units: current * 2^26

# set by test.py to collect a profile
TRACE = False
LAST_EXEC_NS = None
MODE = "hybrid"

_CACHE = {}


def build_kernel(d, th, has_bias, T=T, B_loc=B_LOC, CI=CI, CO=CO):
    TBl = T * B_loc
    n_k = CI // 128
    n_c = CO // 128
    csize = 512
    n_q = TBl // csize  # 2
    t_per_q = csize // B_loc  # 16
    GR = 4  # co-tiles per psum group
    n_g = n_c // GR
    ths = float(th) * SCALE

    nc = bacc.Bacc("TRN2", target_bir_lowering=False, debug=False, num_devices=N_CORES)

    xt = nc.declare_dram_parameter("xt", [128, n_k, TBl], FP16, isOutput=False)
    x8l = nc.declare_dram_parameter("x8l", [128, n_k, TBl], FP8, isOutput=False)
    wt = nc.declare_dram_parameter("wt", [n_c, 128, n_k, 128], FP16, isOutput=False)
    w8l = nc.declare_dram_parameter("w8l", [n_c, 128, n_k, 128], FP8, isOutput=False)
    if has_bias:
        bias = nc.declare_dram_parameter(
            "bias", [n_g, 128, GR, B_loc], FP32, isOutput=False
        )
    spk = nc.declare_dram_parameter("spk", [n_g, 128, GR, TBl], FP16, isOutput=True)

    copy_f = mybir.ActivationFunctionType.Copy

    with tile.TileContext(nc) as tc:
        with (
            tc.tile_pool(name="xp", bufs=1) as x_pool,
            tc.tile_pool(name="wp", bufs=4) as w_pool,
            tc.tile_pool(name="w8p", bufs=4) as w8_pool,
            tc.tile_pool(name="work", bufs=2) as work_pool,
            tc.tile_pool(name="pc", bufs=2, space="PSUM") as pc_pool,
        ):
            XT = x_pool.tile([128, n_k, TBl], FP16)
            # pair dim per k: plane 0 = x~8 (on-chip convert), plane 1 = xl8
            X8 = x_pool.tile([128, n_k, 2, TBl], FP8)

            # head: stream all fp16 data first (XT k-wise, the first four W
            # tiles inserted early) so group 0's k-outer fp16 matmuls track
            # the XT arrivals with all co-tiles available; then the fp8
            # residual stream. x~8/W~8 hi planes convert on ACT as their fp16
            # sources land.
            head_w = {}
            head_w8 = {}
            for c in range(GR):
                head_w[c] = w_pool.tile(
                    [128, n_k, 128], FP16, tag="wt", name=f"w{c}"
                )
                head_w8[c] = w8_pool.tile(
                    [128, 2, n_k, 128], FP8, tag="w8", name=f"w8_{c}"
                )
            nc.sync.dma_start(out=head_w[0][:, 0:4, :], in_=wt[0, :, 0:4, :])
            for k in range(n_k):
                if k == 1:
                    nc.sync.dma_start(out=head_w[1], in_=wt[1, :, :, :])
                elif k == 7:
                    nc.sync.dma_start(out=head_w[2], in_=wt[2, :, :, :])
                elif k == 9:
                    nc.sync.dma_start(out=head_w[3], in_=wt[3, :, :, :])
                if k % 4 == 0 and k > 0:
                    nc.sync.dma_start(
                        out=head_w[0][:, k : k + 4, :], in_=wt[0, :, k : k + 4, :]
                    )
                nc.sync.dma_start(out=XT[:, k, :], in_=xt[:, k, :])
                nc.scalar.activation(
                    X8[:, k, 0, :], XT[:, k, :], copy_f, scale=float(2.0 ** -9)
                )
            for c in range(GR):
                nc.sync.dma_start(out=head_w8[c][:, 0, :, :], in_=w8l[c, :, :, :])
                nc.scalar.activation(
                    head_w8[c][:, 1, :, :], head_w[c], copy_f, scale=float(2.0 ** -3)
                )
                for k in range(8 * c, 8 * c + 8):
                    nc.sync.dma_start(out=X8[:, k, 1, :], in_=x8l[:, k, :])

            for g in range(n_g):
                if g == 0:
                    Wg = {ci: (head_w[ci], head_w8[ci]) for ci in range(GR)}
                else:
                    Wg = {}
                    for ci in range(GR):
                        c = g * GR + ci
                        W_c = w_pool.tile(
                            [128, n_k, 128], FP16, tag="wt", name=f"w{c}"
                        )
                        W8_c = w8_pool.tile(
                            [128, 2, n_k, 128], FP8, tag="w8", name=f"w8_{c}"
                        )
                        nc.sync.dma_start(out=W_c, in_=wt[c, :, :, :])
                        nc.sync.dma_start(out=W8_c[:, 0, :, :], in_=w8l[c, :, :, :])
                        nc.scalar.activation(
                            W8_c[:, 1, :, :], W_c, copy_f, scale=float(2.0 ** -3)
                        )
                        Wg[ci] = (W_c, W8_c)

                if has_bias:
                    b_tile = work_pool.tile([128, GR, B_loc], FP32, tag="bt")
                    nc.sync.dma_start(out=b_tile, in_=bias[g, :, :, :])

                mem = work_pool.tile([128, GR, B_loc], FP32, tag="mem")
                nc.vector.memset(mem, 0.0)

                def rec_steps(tile_, s_tile, t0, nt):
                    for tt in range(nt):
                        oo = tt * B_loc
                        cur = tile_[:, :, oo : oo + B_loc]
                        nc.vector.scalar_tensor_tensor(
                            out=mem,
                            in0=mem,
                            scalar=d,
                            in1=cur,
                            op0=mybir.AluOpType.mult,
                            op1=mybir.AluOpType.add,
                        )
                        if has_bias:
                            nc.vector.tensor_tensor(
                                out=mem, in0=mem, in1=b_tile, op=mybir.AluOpType.add
                            )
                        s_t = s_tile[:, :, oo : oo + B_loc]
                        nc.vector.tensor_scalar(
                            s_t, mem, ths, None, mybir.AluOpType.is_gt
                        )
                        nc.vector.scalar_tensor_tensor(
                            out=mem,
                            in0=s_t,
                            scalar=-ths,
                            in1=mem,
                            op0=mybir.AluOpType.mult,
                            op1=mybir.AluOpType.add,
                        )

                if g == 0:
                    # group 0: two chunk-tiles [128, GR, 512] (each ci slice
                    # owns a full psum bank so accumulation groups interleave
                    # freely), fp16 k-outer in two ci-pair passes to track
                    # the XT DMA stream, then chunk-sequential DR + rec.
                    pcs = [
                        pc_pool.tile([128, GR, csize], FP32, tag="pc", name="pc")
                        for _ in range(n_q)
                    ]
                    # ci 0/1 track the XT stream k-wise; ci 2/3 follow LAG
                    # k-tiles behind (their W tiles arrive a few k-periods in)
                    LAG = 12
                    for kk in range(n_k + LAG):
                        for ci, k in (
                            [(0, kk), (1, kk)] if kk < n_k else []
                        ) + ([(2, kk - LAG), (3, kk - LAG)] if kk >= LAG else []):
                            for q in range(n_q):
                                nc.tensor.matmul(
                                    pcs[q][:, ci, :],
                                    lhsT=Wg[ci][0][:, k, :],
                                    rhs=XT[:, k, q * csize : (q + 1) * csize],
                                    start=(k == 0),
                                    stop=False,
                                )
                    for q in range(n_q):
                        for k in range(n_k):
                            for ci in range(GR):
                                nc.tensor.matmul(
                                    pcs[q][:, ci, :],
                                    lhsT=Wg[ci][1][:, :, k, :],
                                    rhs=X8[:, k, :, q * csize : (q + 1) * csize],
                                    start=False,
                                    stop=(k == n_k - 1),
                                    perf_mode=mybir.MatmulPerfMode.DoubleRow,
                                )
                        s_stage = work_pool.tile(
                            [128, GR, csize], FP16, tag="s", name="s_stage"
                        )
                        rec_steps(pcs[q], s_stage, q * t_per_q, t_per_q)
                        nc.sync.dma_start(
                            out=spk[g, :, :, q * csize : (q + 1) * csize],
                            in_=s_stage,
                        )
                else:
                    # groups 1+: quarter tiles [128, GR, 256] and per-slice
                    # sequential fp16+DR emission (two ci slices share a psum
                    # bank, so slices of a bank must not interleave). Each
                    # quarter's recurrence then overlaps the next quarter's
                    # matmuls, and the final group drains only ~a quarter.
                    qsize = csize // 2

                    def emit_slice(tile_, ci, off, width):
                        for k in range(n_k):
                            nc.tensor.matmul(
                                tile_[:, ci, :],
                                lhsT=Wg[ci][0][:, k, :],
                                rhs=XT[:, k, off : off + width],
                                start=(k == 0),
                                stop=False,
                            )
                        for k in range(n_k):
                            nc.tensor.matmul(
                                tile_[:, ci, :],
                                lhsT=Wg[ci][1][:, :, k, :],
                                rhs=X8[:, k, :, off : off + width],
                                start=False,
                                stop=(k == n_k - 1),
                                perf_mode=mybir.MatmulPerfMode.DoubleRow,
                            )

                    def rec_and_store(tile_, off, width):
                        s_stage = work_pool.tile(
                            [128, GR, width], FP16, tag="s", name="s_stage"
                        )
                        rec_steps(tile_, s_stage, off // B_loc, width // B_loc)
                        nc.sync.dma_start(
                            out=spk[g, :, :, off : off + width], in_=s_stage
                        )

                    n_quart = 2 * n_q if g < n_g - 1 else 2 * n_q - 1
                    for qq in range(n_quart):
                        pcq = pc_pool.tile(
                            [128, GR, qsize], FP32, tag="pc", name="pc"
                        )
                        # [128, 4, 256] spans 2 psum banks (ci 0/1 and 2/3);
                        # alternate banks between consecutive slices so a
                        # group-start's bank-wide has_written clear can never
                        # race the previous slice's in-flight accumulation
                        for ci in (0, 2, 1, 3):
                            emit_slice(pcq, ci, qq * qsize, qsize)
                        rec_and_store(pcq, qq * qsize, qsize)
                    if g == n_g - 1:
                        # final quarter as two single-bank eighth-tiles with
                        # ci interleaved across them (same bank-alternation),
                        # halving the exposed drain of the last recurrence
                        widths = [qsize // 2, qsize // 4, qsize // 8, qsize // 8]
                        offs = [0, qsize // 2, 3 * qsize // 4, 7 * qsize // 8]
                        base = (2 * n_q - 1) * qsize
                        e_tiles = [
                            pc_pool.tile([128, GR, w], FP32, tag="pc", name="pc")
                            for w in widths
                        ]
                        for ci in range(GR):
                            for et, off, w in zip(e_tiles, offs, widths):
                                emit_slice(et, ci, base + off, w)
                        for et, off, w in zip(e_tiles, offs, widths):
                            rec_and_store(et, base + off, w)

    nc.compile()
    return nc


def _f8(a):
    import ml_dtypes

    return np.ascontiguousarray(a).astype(ml_dtypes.float8_e4m3)


def _xt_layout(a):
    """[TB, CI] -> [128, CI//128, TB]: partition p holds ci = k*128+p."""
    TBl, CIl = a.shape
    return np.ascontiguousarray(a.reshape(TBl, CIl // 128, 128).transpose(2, 1, 0))


def _wt_layout(Wm):
    """[CO, CI] -> [CO//128, 128, CI//128, 128]: W[c*128+j, k*128+p] at
    [c, p, k, j]."""
    COl, CIl = Wm.shape
    return np.ascontiguousarray(
        Wm.reshape(COl // 128, 128, CIl // 128, 128).transpose(0, 3, 2, 1)
    )


def kernel(x, W, b, decay, thresh):
    global LAST_EXEC_NS
    x = np.ascontiguousarray(np.asarray(x, dtype=np.float32))
    W = np.ascontiguousarray(np.asarray(W, dtype=np.float32))
    b = np.asarray(b, dtype=np.float32)
    decay = np.asarray(decay, dtype=np.float32)
    thresh = np.asarray(thresh, dtype=np.float32)

    d = float(decay.reshape(-1)[0])
    th = float(thresh.reshape(-1)[0])
    has_bias = bool(np.any(b != 0))

    key = (MODE, d, th, has_bias)
    if key not in _CACHE:
        _CACHE[key] = build_kernel(d, th, has_bias)
    nc = _CACHE[key]

    lim = np.float32(65504.0 * 0.999)

    # weights: shared across cores
    Ws = np.clip(W * S13, -lim, lim)
    Wt = Ws.astype(np.float16)
    Wl = Ws - Wt.astype(np.float32)
    wt_l = _wt_layout(Wt)
    w8l_l = _wt_layout(_f8(Wl * np.float32(2.0 ** 9)))

    in_maps = []
    n_g = (CO // 128) // 4
    for i in range(N_CORES):
        xs = x[:, i * B_LOC : (i + 1) * B_LOC, :].reshape(TB, CI)
        xs = np.clip(xs * S13, -lim, lim)
        xh = xs.astype(np.float16)
        xl = xs - xh.astype(np.float32)
        m = {
            "xt": _xt_layout(xh),
            "x8l": _xt_layout(_f8(xl * np.float32(2.0 ** 3))),
            "wt": wt_l,
            "w8l": w8l_l,
        }
        if has_bias:
            bs = (b * np.float32(SCALE)).astype(np.float32)  # [CO]
            bt = bs.reshape(n_g, 4, 128).transpose(0, 2, 1)  # [n_g, 128, 4]
            m["bias"] = np.ascontiguousarray(
                np.repeat(bt[:, :, :, None], B_LOC, axis=3).astype(np.float32)
            )
        in_maps.append(m)

    res = run_bass_kernel_spmd(
        nc, in_maps, core_ids=list(range(N_CORES)), trace=TRACE
    )
    LAST_EXEC_NS = res.exec_time_ns

    # spikes come back [n_g, 128, GR, TB] fp16 -> [T, B_loc, CO] per core
    outs = []
    for r in res.results:
        s = r["spk"]  # [n_g, 128, GR, TB]
        s = (
            s.reshape(n_g, 128, 4, T, B_LOC)
            .transpose(3, 4, 0, 2, 1)
            .reshape(T, B_LOC, CO)
        )
        outs.append(s.astype(np.float32))
    out = np.concatenate(outs, axis=1)
    return np.ascontiguousarray(out)
